# revision 16
# baseline (speedup 1.0000x reference)
"""Trainium2 Bass kernel: float32 -> 32-channel bit-plane encoding.

For input x [4096, 512] f32, produces out [4096, 512, 32] f32 where
out[b, f, 0] = (x[b,f] < 0) and out[b, f, 1+j] = bit (30-j) of
bitcast_int32(|x[b,f]|), MSB first.

Sharded row-wise over 8 NeuronCores (512 rows each).  Per core:
  pass1 (VectorE):  and_t[p, f, k] = i[p,f] & mask[k]  (masks packed into the
                    input's last columns so one DMA feeds both operands)
  pass2 (ScalarE):  Sign(and_t) -> f32 0/1 bit channels
  pass3 (VectorE):  channel-0 plane = (x < 0) via f32 is_lt on the bitcast
  out-DMA via HWDGE (sync engine).

The out-DMA stream is the bottleneck (~32MB/core at ~400GB/s); the schedule
uses small leading chunks so the DMA stream starts as early as possible and
then stays continuously busy.
"""

import sys

if "/opt/trn_rl_repo" not in sys.path:
    sys.path.insert(0, "/opt/trn_rl_repo")

import numpy as np

import concourse.bass as bass
import concourse.mybir as mybir

P = 128          # SBUF partitions
F = 512          # features per row
K = 32           # output channels per feature
KB = K - 1       # bit channels 1..31
N_CORES = 8
ROWS_TOTAL = 4096
ROWS = ROWS_TOTAL // N_CORES   # rows per core
NRT = ROWS // P                # row tiles per core (4)
XW = F + K                     # packed input width (x columns + 32 mask cols)
FCH_MAX = 256

# Feature chunks per row block: small leading chunks collapse the pipeline
# ramp (first out-DMA starts after ~2us of compute instead of ~16us), and
# small trailing chunks collapse the drain (last Sign+DMA ~2us instead of
# ~12us).
CHUNKS_RB0_SMALL = [32, 32, 64, 128, 256]
CHUNKS_RB_LAST = [256, 128, 64, 32, 32]
CHUNKS_RB = [256, 256]

NBUF_AT = 2
NBUF_OT = 3


def _masks_np() -> np.ndarray:
    # mask column k holds the mask for output channel k; column 0 is a bit30
    # placeholder (its output is overwritten by the pass3 sign compare).
    vals = [1 << 30] + [1 << (31 - k) for k in range(1, K)]
    return np.array(vals, dtype=np.int64).astype(np.uint32).view(np.int32)


def _tiles(small_chunks=True):
    """Yield (t, rt, c_off, c_len) in schedule order."""
    t = 0
    for rt in range(NRT):
        if not small_chunks:
            chunks = CHUNKS_RB
        elif rt == 0:
            chunks = CHUNKS_RB0_SMALL
        elif rt == NRT - 1:
            chunks = CHUNKS_RB_LAST
        else:
            chunks = CHUNKS_RB
        off = 0
        for c_len in chunks:
            yield t, rt, off, c_len
            off += c_len
            t += 1
        assert off == F


def build_nc(strided_sign=True, act_dma=True, warm_act=True,
             small_chunks=True) -> bass.Bass:
    nc = bass.Bass("TRN2", target_bir_lowering=False, debug=False)
    i32, f32 = mybir.dt.int32, mybir.dt.float32

    xm = nc.declare_dram_parameter("xm", [ROWS, XW], i32, isOutput=False)
    out = nc.declare_dram_parameter("out", [ROWS, F * K], f32, isOutput=True)
    xm_ap, out_ap = xm.ap(), out.ap()

    tiles = list(_tiles(small_chunks))
    kch = KB if strided_sign else K   # and-tile channels per feature

    from contextlib import ExitStack
    with ExitStack() as ctx:
        # one xt buffer per row block: no reuse, all four in-DMAs prefetch
        xt = [ctx.enter_context(nc.sbuf_tensor(f"xt{b}", [P, XW], i32))
              for b in range(NRT)]
        at = [ctx.enter_context(nc.sbuf_tensor(f"at{b}", [P, FCH_MAX * kch], i32))
              for b in range(NBUF_AT)]
        ot = [ctx.enter_context(nc.sbuf_tensor(f"ot{b}", [P, FCH_MAX * K], f32))
              for b in range(NBUF_OT)]
        warm = ctx.enter_context(nc.sbuf_tensor("warm", [P, 1], f32))

        in_sem = [ctx.enter_context(nc.semaphore(f"in_sem{b}")) for b in range(NRT)]
        od_sem = [ctx.enter_context(nc.semaphore(f"od_sem{b}"))
                  for b in range(NBUF_OT)]
        tt_sem = ctx.enter_context(nc.semaphore("tt_sem"))
        act_sem = ctx.enter_context(nc.semaphore("act_sem"))
        p3_sem = ctx.enter_context(nc.semaphore("p3_sem"))

        ctx.enter_context(nc.Block())
        block = nc.cur_block

        def p3(vec, t, rt, c_off, c_len):
            """channel-0 plane = (x < 0); on DVE."""
            if t >= NBUF_OT:
                vec.wait_ge(od_sem[t % NBUF_OT], 16 * (t // NBUF_OT))
            xf = xt[rt][:, c_off:c_off + c_len].bitcast(f32)
            sgn = ot[t % NBUF_OT][:, 0:c_len * K] \
                .rearrange("p (f k) -> p f k", k=K)[:, :, 0:1]
            vec.tensor_scalar(
                sgn, xf.unsqueeze(-1), 0.0, None, mybir.AluOpType.is_lt
            ).then_inc(p3_sem)

        @block.vector
        def _(vec: bass.BassEngine):
            seen_rb = -1
            for t, rt, c_off, c_len in tiles:
                if rt != seen_rb:
                    vec.wait_ge(in_sem[rt], 16)
                    seen_rb = rt
                if t >= NBUF_AT:
                    # at[t%NBUF_AT] is free once Sign(t-NBUF_AT) has read it
                    vec.wait_ge(act_sem, t - NBUF_AT + 1)
                moff = F if not strided_sign else F + 1
                in0 = xt[rt][:, c_off:c_off + c_len].unsqueeze(-1) \
                    .broadcast_to([P, c_len, kch])
                in1 = xt[rt][:, moff:moff + kch].unsqueeze(1) \
                    .broadcast_to([P, c_len, kch])
                o3 = at[t % NBUF_AT][:, 0:c_len * kch] \
                    .rearrange("p (f k) -> p f k", k=kch)
                vec.tensor_tensor(
                    o3, in0, in1, mybir.AluOpType.bitwise_and
                ).then_inc(tt_sem)
                if strided_sign:
                    # ot channel 0 is written only by pass3: do it right away
                    p3(vec, t, rt, c_off, c_len)
                else:
                    # ot fully written by Sign; pass3 overwrites channel 0
                    # afterwards -> run pass3 one tile behind
                    if t >= 1:
                        pt, prt, pco, pcl = tiles[t - 1]
                        vec.wait_ge(act_sem, t)
                        p3(vec, pt, prt, pco, pcl)
            if not strided_sign:
                pt, prt, pco, pcl = tiles[-1]
                vec.wait_ge(act_sem, len(tiles))
                p3(vec, pt, prt, pco, pcl)

        @block.scalar
        def _(sc: bass.BassEngine):
            if warm_act:
                # scale=0 -> input is not read (safe on uninitialized SBUF)
                sc.activation(warm[:], warm[:],
                              mybir.ActivationFunctionType.Sign, scale=0.0)
            for t, rt, c_off, c_len in tiles:
                sc.wait_ge(tt_sem, t + 1)
                if t >= NBUF_OT:
                    # ot[t%NBUF_OT] is free once out-DMA(t-NBUF_OT) drained it
                    sc.wait_ge(od_sem[t % NBUF_OT], 16 * (t // NBUF_OT))
                a_in = at[t % NBUF_AT][:, 0:c_len * kch]
                if strided_sign:
                    o_out = ot[t % NBUF_OT][:, 0:c_len * K] \
                        .rearrange("p (f k) -> p f k", k=K)[:, :, 1:K]
                else:
                    o_out = ot[t % NBUF_OT][:, 0:c_len * K]
                sc.activation(
                    o_out, a_in, mybir.ActivationFunctionType.Sign
                ).then_inc(act_sem)

        if not act_dma:
            # unused path retained for A/B: SWDGE in-DMAs on gpsimd
            pass

        @block.sync
        def _(sp: bass.BassEngine):
            # prefetch all row blocks on the SP HWDGE ring before any
            # out-DMA is enqueued (FIFO per ring; these complete first)
            for rt in range(NRT):
                sp.dma_start(
                    xt[rt][:], xm_ap[rt * P:(rt + 1) * P, :]
                ).then_inc(in_sem[rt], 16)
            for t, rt, c_off, c_len in tiles:
                sp.wait_ge(act_sem, t + 1)
                sp.wait_ge(p3_sem, t + 1)
                sp.dma_start(
                    out_ap[rt * P:(rt + 1) * P,
                           c_off * K:(c_off + c_len) * K],
                    ot[t % NBUF_OT][:, 0:c_len * K],
                ).then_inc(od_sem[t % NBUF_OT], 16)

    return nc


_NC_CACHE = None


def _get_nc():
    global _NC_CACHE
    if _NC_CACHE is None:
        _NC_CACHE = build_nc(strided_sign=False, act_dma=False)
    return _NC_CACHE


def pack_shard(x_shard: np.ndarray) -> np.ndarray:
    """[ROWS, F] f32 -> [ROWS, F+K] int32 with mask columns appended."""
    xi = np.ascontiguousarray(x_shard).view(np.int32)
    m = np.broadcast_to(_masks_np(), (x_shard.shape[0], K))
    return np.ascontiguousarray(np.concatenate([xi, m], axis=1))


def kernel(x: np.ndarray) -> np.ndarray:
    from concourse.bass_utils import run_bass_kernel_spmd

    x = np.asarray(x, dtype=np.float32)
    assert x.shape == (ROWS_TOTAL, F), x.shape
    nc = _get_nc()
    in_maps = [
        {"xm": pack_shard(x[i * ROWS:(i + 1) * ROWS])} for i in range(N_CORES)
    ]
    res = run_bass_kernel_spmd(nc, in_maps, list(range(N_CORES)))
    parts = [res.results[i]["out"].reshape(ROWS, F, K) for i in range(N_CORES)]
    return np.concatenate(parts, axis=0)


# revision 19
# speedup vs baseline: 1.0149x; 1.0149x over previous
"""Trainium2 Bass kernel: float32 -> 32-channel bit-plane encoding.

For input x [4096, 512] f32, produces out [4096, 512, 32] f32 where
out[b, f, 0] = (x[b,f] < 0) and out[b, f, 1+j] = bit (30-j) of
bitcast_int32(|x[b,f]|), MSB first.

Sharded row-wise over 8 NeuronCores (512 rows each).  Per core:
  pass1 (VectorE):  and_t[p, f, k] = i[p,f] & mask[k]  (masks packed into the
                    input's last columns so one DMA feeds both operands)
  pass2 (ScalarE):  Sign(and_t) -> f32 0/1 bit channels
  pass3 (VectorE):  channel-0 plane = (x < 0) via f32 is_lt on the bitcast
  out-DMA via HWDGE (sync engine).

The out-DMA stream is the bottleneck (~32MB/core at ~400GB/s); the schedule
uses small leading chunks so the DMA stream starts as early as possible and
then stays continuously busy.
"""

import sys

if "/opt/trn_rl_repo" not in sys.path:
    sys.path.insert(0, "/opt/trn_rl_repo")

import numpy as np

import concourse.bass as bass
import concourse.mybir as mybir

P = 128          # SBUF partitions
F = 512          # features per row
K = 32           # output channels per feature
KB = K - 1       # bit channels 1..31
N_CORES = 8
ROWS_TOTAL = 4096
ROWS = ROWS_TOTAL // N_CORES   # rows per core
NRT = ROWS // P                # row tiles per core (4)
XW = F + K                     # packed input width (x columns + 32 mask cols)
FCH_MAX = 256

# Feature chunks per row block: small leading chunks collapse the pipeline
# ramp (first out-DMA starts after ~2us of compute instead of ~16us), and
# small trailing chunks collapse the drain (last Sign+DMA ~2us instead of
# ~12us).
CHUNKS_RB0_SMALL = [32, 32, 64, 128, 256]
CHUNKS_RB_LAST = [256, 128, 64, 32, 32]
CHUNKS_RB = [256, 256]

NBUF_AT = 2
NBUF_OT = 3


def _masks_np() -> np.ndarray:
    # mask column k holds the mask for output channel k; column 0 is a bit30
    # placeholder (its output is overwritten by the pass3 sign compare).
    vals = [1 << 30] + [1 << (31 - k) for k in range(1, K)]
    return np.array(vals, dtype=np.int64).astype(np.uint32).view(np.int32)


def _tiles(small_chunks=True):
    """Yield (t, rt, c_off, c_len) in schedule order."""
    t = 0
    for rt in range(NRT):
        if not small_chunks:
            chunks = CHUNKS_RB
        elif rt == 0:
            chunks = CHUNKS_RB0_SMALL
        elif rt == NRT - 1:
            chunks = CHUNKS_RB_LAST
        else:
            chunks = CHUNKS_RB
        off = 0
        for c_len in chunks:
            yield t, rt, off, c_len
            off += c_len
            t += 1
        assert off == F


def build_nc(strided_sign=True, in_dma="gp", warm_act=True,
             small_chunks=True) -> bass.Bass:
    nc = bass.Bass("TRN2", target_bir_lowering=False, debug=False)
    i32, f32 = mybir.dt.int32, mybir.dt.float32

    xm = nc.declare_dram_parameter("xm", [ROWS, XW], i32, isOutput=False)
    out = nc.declare_dram_parameter("out", [ROWS, F * K], f32, isOutput=True)
    xm_ap, out_ap = xm.ap(), out.ap()

    tiles = list(_tiles(small_chunks))
    kch = KB if strided_sign else K   # and-tile channels per feature

    from contextlib import ExitStack
    with ExitStack() as ctx:
        # one xt buffer per row block: no reuse, all four in-DMAs prefetch
        xt = [ctx.enter_context(nc.sbuf_tensor(f"xt{b}", [P, XW], i32))
              for b in range(NRT)]
        at = [ctx.enter_context(nc.sbuf_tensor(f"at{b}", [P, FCH_MAX * kch], i32))
              for b in range(NBUF_AT)]
        ot = [ctx.enter_context(nc.sbuf_tensor(f"ot{b}", [P, FCH_MAX * K], f32))
              for b in range(NBUF_OT)]
        warm = ctx.enter_context(nc.sbuf_tensor("warm", [P, 1], f32))

        in_sem = [ctx.enter_context(nc.semaphore(f"in_sem{b}")) for b in range(NRT)]
        od_sem = [ctx.enter_context(nc.semaphore(f"od_sem{b}"))
                  for b in range(NBUF_OT)]
        tt_sem = ctx.enter_context(nc.semaphore("tt_sem"))
        act_sem = ctx.enter_context(nc.semaphore("act_sem"))
        p3_sem = ctx.enter_context(nc.semaphore("p3_sem"))

        ctx.enter_context(nc.Block())
        block = nc.cur_block

        def p3(vec, t, rt, c_off, c_len):
            """channel-0 plane = (x < 0); on DVE."""
            if t >= NBUF_OT:
                vec.wait_ge(od_sem[t % NBUF_OT], 16 * (t // NBUF_OT))
            xf = xt[rt][:, c_off:c_off + c_len].bitcast(f32)
            sgn = ot[t % NBUF_OT][:, 0:c_len * K] \
                .rearrange("p (f k) -> p f k", k=K)[:, :, 0:1]
            vec.tensor_scalar(
                sgn, xf.unsqueeze(-1), 0.0, None, mybir.AluOpType.is_lt
            ).then_inc(p3_sem)

        @block.vector
        def _(vec: bass.BassEngine):
            seen_rb = -1
            for t, rt, c_off, c_len in tiles:
                if rt != seen_rb:
                    vec.wait_ge(in_sem[rt], 16)
                    seen_rb = rt
                if t >= NBUF_AT:
                    # at[t%NBUF_AT] is free once Sign(t-NBUF_AT) has read it
                    vec.wait_ge(act_sem, t - NBUF_AT + 1)
                moff = F if not strided_sign else F + 1
                in0 = xt[rt][:, c_off:c_off + c_len].unsqueeze(-1) \
                    .broadcast_to([P, c_len, kch])
                in1 = xt[rt][:, moff:moff + kch].unsqueeze(1) \
                    .broadcast_to([P, c_len, kch])
                o3 = at[t % NBUF_AT][:, 0:c_len * kch] \
                    .rearrange("p (f k) -> p f k", k=kch)
                vec.tensor_tensor(
                    o3, in0, in1, mybir.AluOpType.bitwise_and
                ).then_inc(tt_sem)
                if strided_sign:
                    # ot channel 0 is written only by pass3: do it right away
                    p3(vec, t, rt, c_off, c_len)
                else:
                    # ot fully written by Sign; pass3 overwrites channel 0
                    # afterwards -> run pass3 one tile behind
                    if t >= 1:
                        pt, prt, pco, pcl = tiles[t - 1]
                        vec.wait_ge(act_sem, t)
                        p3(vec, pt, prt, pco, pcl)
            if not strided_sign:
                pt, prt, pco, pcl = tiles[-1]
                vec.wait_ge(act_sem, len(tiles))
                p3(vec, pt, prt, pco, pcl)

        @block.scalar
        def _(sc: bass.BassEngine):
            if warm_act:
                # scale=0 -> input is not read (safe on uninitialized SBUF)
                sc.activation(warm[:], warm[:],
                              mybir.ActivationFunctionType.Sign, scale=0.0)
            for t, rt, c_off, c_len in tiles:
                sc.wait_ge(tt_sem, t + 1)
                if t >= NBUF_OT:
                    # ot[t%NBUF_OT] is free once out-DMA(t-NBUF_OT) drained it
                    sc.wait_ge(od_sem[t % NBUF_OT], 16 * (t // NBUF_OT))
                a_in = at[t % NBUF_AT][:, 0:c_len * kch]
                if strided_sign:
                    o_out = ot[t % NBUF_OT][:, 0:c_len * K] \
                        .rearrange("p (f k) -> p f k", k=K)[:, :, 1:K]
                else:
                    o_out = ot[t % NBUF_OT][:, 0:c_len * K]
                sc.activation(
                    o_out, a_in, mybir.ActivationFunctionType.Sign
                ).then_inc(act_sem)

        if in_dma == "gp":
            @block.gpsimd
            def _(gp: bass.BassEngine):
                for rt in range(NRT):
                    gp.dma_start(
                        xt[rt][:], xm_ap[rt * P:(rt + 1) * P, :]
                    ).then_inc(in_sem[rt], 16)

        @block.sync
        def _(sp: bass.BassEngine):
            if in_dma == "sp":
                # prefetch all row blocks on the SP HWDGE ring before any
                # out-DMA is enqueued (FIFO per ring; these complete first)
                for rt in range(NRT):
                    sp.dma_start(
                        xt[rt][:], xm_ap[rt * P:(rt + 1) * P, :]
                    ).then_inc(in_sem[rt], 16)
            for t, rt, c_off, c_len in tiles:
                sp.wait_ge(act_sem, t + 1)
                sp.wait_ge(p3_sem, t + 1)
                sp.dma_start(
                    out_ap[rt * P:(rt + 1) * P,
                           c_off * K:(c_off + c_len) * K],
                    ot[t % NBUF_OT][:, 0:c_len * K],
                ).then_inc(od_sem[t % NBUF_OT], 16)

    return nc


_NC_CACHE = None


def _get_nc():
    global _NC_CACHE
    if _NC_CACHE is None:
        _NC_CACHE = build_nc(strided_sign=False, in_dma="gp")
    return _NC_CACHE


def pack_shard(x_shard: np.ndarray) -> np.ndarray:
    """[ROWS, F] f32 -> [ROWS, F+K] int32 with mask columns appended."""
    xi = np.ascontiguousarray(x_shard).view(np.int32)
    m = np.broadcast_to(_masks_np(), (x_shard.shape[0], K))
    return np.ascontiguousarray(np.concatenate([xi, m], axis=1))


def kernel(x: np.ndarray) -> np.ndarray:
    from concourse.bass_utils import run_bass_kernel_spmd

    x = np.asarray(x, dtype=np.float32)
    assert x.shape == (ROWS_TOTAL, F), x.shape
    nc = _get_nc()
    in_maps = [
        {"xm": pack_shard(x[i * ROWS:(i + 1) * ROWS])} for i in range(N_CORES)
    ]
    res = run_bass_kernel_spmd(nc, in_maps, list(range(N_CORES)))
    parts = [res.results[i]["out"].reshape(ROWS, F, K) for i in range(N_CORES)]
    return np.concatenate(parts, axis=0)


# revision 23
# speedup vs baseline: 1.1825x; 1.1652x over previous
"""Trainium2 Bass kernel: float32 -> 32-channel bit-plane encoding.

For input x [4096, 512] f32, produces out [4096, 512, 32] f32 where
out[b, f, 0] = (x[b,f] < 0) and out[b, f, 1+j] = bit (30-j) of
bitcast_int32(|x[b,f]|), MSB first.

Sharded row-wise over 8 NeuronCores (512 rows each).  Per core:
  pass1 (VectorE):  and_t[p, f, k] = i[p,f] & mask[k]  (masks packed into the
                    input's last columns so one DMA feeds both operands)
  pass2 (ScalarE):  Sign(and_t) -> f32 0/1 bit channels
  pass3 (VectorE):  channel-0 plane = (x < 0) via f32 is_lt on the bitcast
  out-DMA via HWDGE (sync engine).

The out-DMA stream is the bottleneck (~32MB/core at ~400GB/s); the schedule
uses small leading chunks so the DMA stream starts as early as possible and
then stays continuously busy.
"""

import sys

if "/opt/trn_rl_repo" not in sys.path:
    sys.path.insert(0, "/opt/trn_rl_repo")

import numpy as np

import concourse.bass as bass
import concourse.mybir as mybir

P = 128          # SBUF partitions
F = 512          # features per row
K = 32           # output channels per feature
KB = K - 1       # bit channels 1..31
N_CORES = 8
ROWS_TOTAL = 4096
ROWS = ROWS_TOTAL // N_CORES   # rows per core
NRT = ROWS // P                # row tiles per core (4)
XW = F + K                     # packed input width (x columns + 32 mask cols)
FCH_MAX = 256

# Feature chunks per row block: small leading chunks collapse the pipeline
# ramp (first out-DMA starts after ~2us of compute instead of ~16us), and
# small trailing chunks collapse the drain (last Sign+DMA ~2us instead of
# ~12us).
CHUNKS_RB0_SMALL = [32, 32, 64, 128, 256]
CHUNKS_RB = [256, 256]

NBUF_AT = 2
NBUF_OT = 3


def _masks_np() -> np.ndarray:
    # mask column k holds the mask for output channel k; column 0 is a bit30
    # placeholder (its output is overwritten by the pass3 sign compare).
    vals = [1 << 30] + [1 << (31 - k) for k in range(1, K)]
    return np.array(vals, dtype=np.int64).astype(np.uint32).view(np.int32)


def _tiles(small_chunks=True):
    """Yield (t, rt, c_off, c_len) in schedule order."""
    t = 0
    for rt in range(NRT):
        chunks = CHUNKS_RB0_SMALL if (rt == 0 and small_chunks) else CHUNKS_RB
        off = 0
        for c_len in chunks:
            yield t, rt, off, c_len
            off += c_len
            t += 1
        assert off == F


def build_nc(strided_sign=True, in_dma="gp", warm_act=True,
             small_chunks=True) -> bass.Bass:
    nc = bass.Bass("TRN2", target_bir_lowering=False, debug=False)
    i32, f32 = mybir.dt.int32, mybir.dt.float32

    xm = nc.declare_dram_parameter("xm", [ROWS, XW], i32, isOutput=False)
    out = nc.declare_dram_parameter("out", [ROWS, F * K], f32, isOutput=True)
    xm_ap, out_ap = xm.ap(), out.ap()

    tiles = list(_tiles(small_chunks))
    kch = KB if strided_sign else K   # and-tile channels per feature

    from contextlib import ExitStack
    with ExitStack() as ctx:
        # one xt buffer per row block: no reuse, all four in-DMAs prefetch
        xt = [ctx.enter_context(nc.sbuf_tensor(f"xt{b}", [P, XW], i32))
              for b in range(NRT)]
        at = [ctx.enter_context(nc.sbuf_tensor(f"at{b}", [P, FCH_MAX * kch], i32))
              for b in range(NBUF_AT)]
        ot = [ctx.enter_context(nc.sbuf_tensor(f"ot{b}", [P, FCH_MAX * K], f32))
              for b in range(NBUF_OT)]
        warm = ctx.enter_context(nc.sbuf_tensor("warm", [P, 1], f32))

        in_sem = [ctx.enter_context(nc.semaphore(f"in_sem{b}")) for b in range(NRT)]
        od_sem = [ctx.enter_context(nc.semaphore(f"od_sem{b}"))
                  for b in range(NBUF_OT)]
        tt_sem = ctx.enter_context(nc.semaphore("tt_sem"))
        act_sem = ctx.enter_context(nc.semaphore("act_sem"))
        p3_sem = ctx.enter_context(nc.semaphore("p3_sem"))

        ctx.enter_context(nc.Block())
        block = nc.cur_block

        def p3(vec, t, rt, c_off, c_len):
            """channel-0 plane = (x < 0); on DVE."""
            if t >= NBUF_OT:
                vec.wait_ge(od_sem[t % NBUF_OT], 16 * (t // NBUF_OT))
            xf = xt[rt][:, c_off:c_off + c_len].bitcast(f32)
            sgn = ot[t % NBUF_OT][:, 0:c_len * K] \
                .rearrange("p (f k) -> p f k", k=K)[:, :, 0:1]
            vec.tensor_scalar(
                sgn, xf.unsqueeze(-1), 0.0, None, mybir.AluOpType.is_lt
            ).then_inc(p3_sem)

        @block.vector
        def _(vec: bass.BassEngine):
            seen_rb = -1
            for t, rt, c_off, c_len in tiles:
                if rt != seen_rb:
                    vec.wait_ge(in_sem[rt], 16)
                    seen_rb = rt
                if t >= NBUF_AT:
                    # at[t%NBUF_AT] is free once Sign(t-NBUF_AT) has read it
                    vec.wait_ge(act_sem, t - NBUF_AT + 1)
                moff = F if not strided_sign else F + 1
                in0 = xt[rt][:, c_off:c_off + c_len].unsqueeze(-1) \
                    .broadcast_to([P, c_len, kch])
                in1 = xt[rt][:, moff:moff + kch].unsqueeze(1) \
                    .broadcast_to([P, c_len, kch])
                o3 = at[t % NBUF_AT][:, 0:c_len * kch] \
                    .rearrange("p (f k) -> p f k", k=kch)
                vec.tensor_tensor(
                    o3, in0, in1, mybir.AluOpType.bitwise_and
                ).then_inc(tt_sem)
                # pass3 strictly after Sign(t-1) (concurrent DVE+ACT writes
                # to one ot tile corrupt results on HW) -> one tile behind
                if t >= 1:
                    pt, prt, pco, pcl = tiles[t - 1]
                    vec.wait_ge(act_sem, t)
                    p3(vec, pt, prt, pco, pcl)
            pt, prt, pco, pcl = tiles[-1]
            vec.wait_ge(act_sem, len(tiles))
            p3(vec, pt, prt, pco, pcl)

        @block.scalar
        def _(sc: bass.BassEngine):
            if warm_act:
                # scale=0 -> input is not read (safe on uninitialized SBUF)
                sc.activation(warm[:], warm[:],
                              mybir.ActivationFunctionType.Sign, scale=0.0)
            for t, rt, c_off, c_len in tiles:
                sc.wait_ge(tt_sem, t + 1)
                if t >= NBUF_OT:
                    # ot[t%NBUF_OT] is free once out-DMA(t-NBUF_OT) drained it
                    sc.wait_ge(od_sem[t % NBUF_OT], 16 * (t // NBUF_OT))
                a_in = at[t % NBUF_AT][:, 0:c_len * kch]
                if strided_sign:
                    o_out = ot[t % NBUF_OT][:, 0:c_len * K] \
                        .rearrange("p (f k) -> p f k", k=K)[:, :, 1:K]
                else:
                    o_out = ot[t % NBUF_OT][:, 0:c_len * K]
                sc.activation(
                    o_out, a_in, mybir.ActivationFunctionType.Sign
                ).then_inc(act_sem)

        if in_dma == "gp":
            @block.gpsimd
            def _(gp: bass.BassEngine):
                for rt in range(NRT):
                    gp.dma_start(
                        xt[rt][:], xm_ap[rt * P:(rt + 1) * P, :]
                    ).then_inc(in_sem[rt], 16)

        @block.sync
        def _(sp: bass.BassEngine):
            if in_dma == "sp":
                # prefetch all row blocks on the SP HWDGE ring before any
                # out-DMA is enqueued (FIFO per ring; these complete first)
                for rt in range(NRT):
                    sp.dma_start(
                        xt[rt][:], xm_ap[rt * P:(rt + 1) * P, :]
                    ).then_inc(in_sem[rt], 16)
            for t, rt, c_off, c_len in tiles:
                sp.wait_ge(act_sem, t + 1)
                sp.wait_ge(p3_sem, t + 1)
                sp.dma_start(
                    out_ap[rt * P:(rt + 1) * P,
                           c_off * K:(c_off + c_len) * K],
                    ot[t % NBUF_OT][:, 0:c_len * K],
                ).then_inc(od_sem[t % NBUF_OT], 16)

    return nc


_NC_CACHE = None


def _get_nc():
    global _NC_CACHE
    if _NC_CACHE is None:
        _NC_CACHE = build_nc(strided_sign=True, in_dma="gp")
    return _NC_CACHE


def pack_shard(x_shard: np.ndarray) -> np.ndarray:
    """[ROWS, F] f32 -> [ROWS, F+K] int32 with mask columns appended."""
    xi = np.ascontiguousarray(x_shard).view(np.int32)
    m = np.broadcast_to(_masks_np(), (x_shard.shape[0], K))
    return np.ascontiguousarray(np.concatenate([xi, m], axis=1))


def kernel(x: np.ndarray) -> np.ndarray:
    from concourse.bass_utils import run_bass_kernel_spmd

    x = np.asarray(x, dtype=np.float32)
    assert x.shape == (ROWS_TOTAL, F), x.shape
    nc = _get_nc()
    in_maps = [
        {"xm": pack_shard(x[i * ROWS:(i + 1) * ROWS])} for i in range(N_CORES)
    ]
    res = run_bass_kernel_spmd(nc, in_maps, list(range(N_CORES)))
    parts = [res.results[i]["out"].reshape(ROWS, F, K) for i in range(N_CORES)]
    return np.concatenate(parts, axis=0)


# revision 24
# speedup vs baseline: 1.1845x; 1.0016x over previous
"""Trainium2 Bass kernel: float32 -> 32-channel bit-plane encoding.

For input x [4096, 512] f32, produces out [4096, 512, 32] f32 where
out[b, f, 0] = (x[b,f] < 0) and out[b, f, 1+j] = bit (30-j) of
bitcast_int32(|x[b,f]|), MSB first.

Host-side repack makes every channel a uniform positive-mask bit test:
  i' = (bitcast_i32(x) & 0x7FFFFFFF) | ((x < 0) << 31)
so channel k is Sign(uint32(i' & mask[k])) with mask[0] = 0x80000000 and
mask[k] = 1 << (31-k).  (bits 30..0 of x equal those of |x|, and replacing
bit 31 with the float compare keeps -0.0 / NaN semantics exact.)

Sharded row-wise over 8 NeuronCores (512 rows each).  Per core:
  pass1 (VectorE):  and_t[p, f, k] = i'[p,f] & mask[k]   (uint32; masks are
                    packed into the input's last 32 columns so one DMA feeds
                    both operands)
  pass2 (ScalarE):  out = Sign(and_t)  (uint32 -> f32: {0, 2^s} -> {0.0, 1.0})
  out-DMA via HWDGE (sync engine); in-DMAs via SWDGE (gpsimd).

The schedule is a 3-stage pipeline over feature chunks; small leading chunks
start the out-DMA stream early, and the stream then stays continuously busy
(~32MB/core at ~430GB/s is the bottleneck).
"""

import sys

if "/opt/trn_rl_repo" not in sys.path:
    sys.path.insert(0, "/opt/trn_rl_repo")

import numpy as np

import concourse.bass as bass
import concourse.mybir as mybir

P = 128          # SBUF partitions
F = 512          # features per row
K = 32           # output channels per feature
N_CORES = 8
ROWS_TOTAL = 4096
ROWS = ROWS_TOTAL // N_CORES   # rows per core
NRT = ROWS // P                # row tiles per core (4)
XW = F + K                     # packed input width (x columns + 32 mask cols)
FCH_MAX = 256

# Feature chunks per row block: small leading chunks collapse the ramp.
CHUNKS_RB0 = [32, 32, 64, 128, 256]
CHUNKS_RB = [256, 256]
CHUNKS_RB_LAST = [256, 128, 128]   # smaller tail -> shorter final Sign+DMA

NBUF_AT = 2
NBUF_OT = 4


def _masks_np() -> np.ndarray:
    vals = [1 << (31 - k) for k in range(K)]   # k=0 -> 0x80000000
    return np.array(vals, dtype=np.int64).astype(np.uint32).view(np.int32)


def _tiles(small_chunks=True, tail_split=True):
    """Yield (t, rt, c_off, c_len) in schedule order."""
    t = 0
    for rt in range(NRT):
        if rt == 0 and small_chunks:
            chunks = CHUNKS_RB0
        elif rt == NRT - 1 and tail_split:
            chunks = CHUNKS_RB_LAST
        else:
            chunks = CHUNKS_RB
        off = 0
        for c_len in chunks:
            yield t, rt, off, c_len
            off += c_len
            t += 1
        assert off == F


def build_nc(in_dma="gp", warm_act=True, small_chunks=True,
             tail_split=True, nbuf_ot=NBUF_OT) -> bass.Bass:
    nc = bass.Bass("TRN2", target_bir_lowering=False, debug=False)
    i32, f32, u32 = mybir.dt.int32, mybir.dt.float32, mybir.dt.uint32

    xm = nc.declare_dram_parameter("xm", [ROWS, XW], i32, isOutput=False)
    out = nc.declare_dram_parameter("out", [ROWS, F * K], f32, isOutput=True)
    xm_ap, out_ap = xm.ap(), out.ap()

    tiles = list(_tiles(small_chunks, tail_split))

    from contextlib import ExitStack
    with ExitStack() as ctx:
        # one xt buffer per row block: no reuse, all four in-DMAs prefetch
        xt = [ctx.enter_context(nc.sbuf_tensor(f"xt{b}", [P, XW], i32))
              for b in range(NRT)]
        at = [ctx.enter_context(nc.sbuf_tensor(f"at{b}", [P, FCH_MAX * K], u32))
              for b in range(NBUF_AT)]
        ot = [ctx.enter_context(nc.sbuf_tensor(f"ot{b}", [P, FCH_MAX * K], f32))
              for b in range(nbuf_ot)]
        warm = ctx.enter_context(nc.sbuf_tensor("warm", [P, 1], f32))

        in_sem = [ctx.enter_context(nc.semaphore(f"in_sem{b}")) for b in range(NRT)]
        od_sem = [ctx.enter_context(nc.semaphore(f"od_sem{b}"))
                  for b in range(nbuf_ot)]
        tt_sem = ctx.enter_context(nc.semaphore("tt_sem"))
        act_sem = ctx.enter_context(nc.semaphore("act_sem"))

        ctx.enter_context(nc.Block())
        block = nc.cur_block

        @block.vector
        def _(vec: bass.BassEngine):
            seen_rb = -1
            for t, rt, c_off, c_len in tiles:
                if rt != seen_rb:
                    vec.wait_ge(in_sem[rt], 16)
                    seen_rb = rt
                if t >= NBUF_AT:
                    # at[t%NBUF_AT] is free once Sign(t-NBUF_AT) has read it
                    vec.wait_ge(act_sem, t - NBUF_AT + 1)
                in0 = xt[rt][:, c_off:c_off + c_len].bitcast(u32) \
                    .unsqueeze(-1).broadcast_to([P, c_len, K])
                in1 = xt[rt][:, F:F + K].bitcast(u32) \
                    .unsqueeze(1).broadcast_to([P, c_len, K])
                o3 = at[t % NBUF_AT][:, 0:c_len * K] \
                    .rearrange("p (f k) -> p f k", k=K)
                vec.tensor_tensor(
                    o3, in0, in1, mybir.AluOpType.bitwise_and
                ).then_inc(tt_sem)

        @block.scalar
        def _(sc: bass.BassEngine):
            if warm_act:
                # scale=0 -> input is not read (safe on uninitialized SBUF)
                sc.activation(warm[:], warm[:],
                              mybir.ActivationFunctionType.Sign, scale=0.0)
            for t, rt, c_off, c_len in tiles:
                sc.wait_ge(tt_sem, t + 1)
                if t >= nbuf_ot:
                    # ot[t%nbuf_ot] is free once out-DMA(t-nbuf_ot) drained it
                    sc.wait_ge(od_sem[t % nbuf_ot], 16 * (t // nbuf_ot))
                sc.activation(
                    ot[t % nbuf_ot][:, 0:c_len * K],
                    at[t % NBUF_AT][:, 0:c_len * K],
                    mybir.ActivationFunctionType.Sign,
                ).then_inc(act_sem)

        if in_dma == "gp":
            @block.gpsimd
            def _(gp: bass.BassEngine):
                for rt in range(NRT):
                    gp.dma_start(
                        xt[rt][:], xm_ap[rt * P:(rt + 1) * P, :]
                    ).then_inc(in_sem[rt], 16)

        @block.sync
        def _(sp: bass.BassEngine):
            if in_dma == "sp":
                for rt in range(NRT):
                    sp.dma_start(
                        xt[rt][:], xm_ap[rt * P:(rt + 1) * P, :]
                    ).then_inc(in_sem[rt], 16)
            for t, rt, c_off, c_len in tiles:
                sp.wait_ge(act_sem, t + 1)
                sp.dma_start(
                    out_ap[rt * P:(rt + 1) * P,
                           c_off * K:(c_off + c_len) * K],
                    ot[t % nbuf_ot][:, 0:c_len * K],
                ).then_inc(od_sem[t % nbuf_ot], 16)

    return nc


_NC_CACHE = None


def _get_nc():
    global _NC_CACHE
    if _NC_CACHE is None:
        _NC_CACHE = build_nc()
    return _NC_CACHE


def pack_shard(x_shard: np.ndarray) -> np.ndarray:
    """[ROWS, F] f32 -> [ROWS, F+K] int32: sign-normalized bitcast columns
    plus the 32 mask columns."""
    x_shard = np.ascontiguousarray(x_shard)
    xi = x_shard.view(np.uint32)
    xi = (xi & np.uint32(0x7FFFFFFF)) | \
        ((x_shard < 0).astype(np.uint32) << np.uint32(31))
    m = np.broadcast_to(_masks_np(), (x_shard.shape[0], K))
    return np.ascontiguousarray(
        np.concatenate([xi.view(np.int32), m], axis=1))


def kernel(x: np.ndarray) -> np.ndarray:
    from concourse.bass_utils import run_bass_kernel_spmd

    x = np.asarray(x, dtype=np.float32)
    assert x.shape == (ROWS_TOTAL, F), x.shape
    nc = _get_nc()
    in_maps = [
        {"xm": pack_shard(x[i * ROWS:(i + 1) * ROWS])} for i in range(N_CORES)
    ]
    res = run_bass_kernel_spmd(nc, in_maps, list(range(N_CORES)))
    parts = [res.results[i]["out"].reshape(ROWS, F, K) for i in range(N_CORES)]
    return np.concatenate(parts, axis=0)


# revision 25
# speedup vs baseline: 1.1907x; 1.0053x over previous
"""Trainium2 Bass kernel: float32 -> 32-channel bit-plane encoding.

For input x [4096, 512] f32, produces out [4096, 512, 32] f32 where
out[b, f, 0] = (x[b,f] < 0) and out[b, f, 1+j] = bit (30-j) of
bitcast_int32(|x[b,f]|), MSB first.

Host-side repack makes every channel a uniform positive-mask bit test:
  i' = (bitcast_i32(x) & 0x7FFFFFFF) | ((x < 0) << 31)
so channel k is Sign(uint32(i' & mask[k])) with mask[0] = 0x80000000 and
mask[k] = 1 << (31-k).  (bits 30..0 of x equal those of |x|, and replacing
bit 31 with the float compare keeps -0.0 / NaN semantics exact.)

Sharded row-wise over 8 NeuronCores (512 rows each).  Per core:
  pass1 (VectorE):  and_t[p, f, k] = i'[p,f] & mask[k]   (uint32; masks are
                    packed into the input's last 32 columns so one DMA feeds
                    both operands)
  pass2 (ScalarE):  out = Sign(and_t)  (uint32 -> f32: {0, 2^s} -> {0.0, 1.0})
  out-DMA via HWDGE (sync engine); in-DMAs via SWDGE (gpsimd).

The schedule is a 3-stage pipeline over feature chunks; small leading chunks
start the out-DMA stream early, and the stream then stays continuously busy
(~32MB/core at ~430GB/s is the bottleneck).
"""

import sys

if "/opt/trn_rl_repo" not in sys.path:
    sys.path.insert(0, "/opt/trn_rl_repo")

import numpy as np

import concourse.bass as bass
import concourse.mybir as mybir

P = 128          # SBUF partitions
F = 512          # features per row
K = 32           # output channels per feature
N_CORES = 8
ROWS_TOTAL = 4096
ROWS = ROWS_TOTAL // N_CORES   # rows per core
NRT = ROWS // P                # row tiles per core (4)
XW = F + K                     # packed input width (x columns + 32 mask cols)
FCH_MAX = 256

# Feature chunks per row block: small leading chunks collapse the ramp.
CHUNKS_RB0 = [32, 32, 64, 128, 256]
CHUNKS_RB = [256, 256]
CHUNKS_RB_LAST = [256, 128, 128]   # smaller tail -> shorter final Sign+DMA

NBUF_AT = 2
NBUF_OT = 4


def _masks_np() -> np.ndarray:
    vals = [1 << (31 - k) for k in range(K)]   # k=0 -> 0x80000000
    return np.array(vals, dtype=np.int64).astype(np.uint32).view(np.int32)


def _tiles(small_chunks=True, tail_split=True):
    """Yield (t, rt, c_off, c_len) in schedule order."""
    t = 0
    for rt in range(NRT):
        if rt == 0 and small_chunks:
            chunks = CHUNKS_RB0
        elif rt == NRT - 1 and tail_split:
            chunks = CHUNKS_RB_LAST
        else:
            chunks = CHUNKS_RB
        off = 0
        for c_len in chunks:
            yield t, rt, off, c_len
            off += c_len
            t += 1
        assert off == F


def build_nc(in_dma="gp", warm_act=True, small_chunks=True,
             tail_split=True, nbuf_ot=NBUF_OT) -> bass.Bass:
    nc = bass.Bass("TRN2", target_bir_lowering=False, debug=False)
    i32, f32, u32 = mybir.dt.int32, mybir.dt.float32, mybir.dt.uint32

    xm = nc.declare_dram_parameter("xm", [ROWS, XW], i32, isOutput=False)
    out = nc.declare_dram_parameter("out", [ROWS, F * K], f32, isOutput=True)
    xm_ap, out_ap = xm.ap(), out.ap()

    tiles = list(_tiles(small_chunks, tail_split))

    from contextlib import ExitStack
    with ExitStack() as ctx:
        # one xt buffer per row block: no reuse, all four in-DMAs prefetch
        xt = [ctx.enter_context(nc.sbuf_tensor(f"xt{b}", [P, XW], i32))
              for b in range(NRT)]
        at = [ctx.enter_context(nc.sbuf_tensor(f"at{b}", [P, FCH_MAX * K], u32))
              for b in range(NBUF_AT)]
        ot = [ctx.enter_context(nc.sbuf_tensor(f"ot{b}", [P, FCH_MAX * K], f32))
              for b in range(nbuf_ot)]
        warm = ctx.enter_context(nc.sbuf_tensor("warm", [P, 1], f32))

        in_sem = [ctx.enter_context(nc.semaphore(f"in_sem{b}")) for b in range(NRT)]
        od_sem = [ctx.enter_context(nc.semaphore(f"od_sem{b}"))
                  for b in range(nbuf_ot)]
        tt_sem = ctx.enter_context(nc.semaphore("tt_sem"))
        act_sem = ctx.enter_context(nc.semaphore("act_sem"))

        ctx.enter_context(nc.Block())
        block = nc.cur_block

        @block.vector
        def _(vec: bass.BassEngine):
            seen_rb = -1
            for t, rt, c_off, c_len in tiles:
                if rt != seen_rb:
                    vec.wait_ge(in_sem[rt], 16)
                    seen_rb = rt
                if t >= NBUF_AT:
                    # at[t%NBUF_AT] is free once Sign(t-NBUF_AT) has read it
                    vec.wait_ge(act_sem, t - NBUF_AT + 1)
                in0 = xt[rt][:, c_off:c_off + c_len].bitcast(u32) \
                    .unsqueeze(-1).broadcast_to([P, c_len, K])
                in1 = xt[rt][:, F:F + K].bitcast(u32) \
                    .unsqueeze(1).broadcast_to([P, c_len, K])
                o3 = at[t % NBUF_AT][:, 0:c_len * K] \
                    .rearrange("p (f k) -> p f k", k=K)
                vec.tensor_tensor(
                    o3, in0, in1, mybir.AluOpType.bitwise_and
                ).then_inc(tt_sem)

        @block.scalar
        def _(sc: bass.BassEngine):
            if warm_act:
                # scale=0 -> input is not read (safe on uninitialized SBUF)
                sc.activation(warm[:], warm[:],
                              mybir.ActivationFunctionType.Sign, scale=0.0)
            for t, rt, c_off, c_len in tiles:
                sc.wait_ge(tt_sem, t + 1)
                if t >= nbuf_ot:
                    # ot[t%nbuf_ot] is free once out-DMA(t-nbuf_ot) drained it
                    sc.wait_ge(od_sem[t % nbuf_ot], 16 * (t // nbuf_ot))
                sc.activation(
                    ot[t % nbuf_ot][:, 0:c_len * K],
                    at[t % NBUF_AT][:, 0:c_len * K],
                    mybir.ActivationFunctionType.Sign,
                ).then_inc(act_sem)

        if in_dma == "gp":
            @block.gpsimd
            def _(gp: bass.BassEngine):
                for rt in range(NRT):
                    gp.dma_start(
                        xt[rt][:], xm_ap[rt * P:(rt + 1) * P, :]
                    ).then_inc(in_sem[rt], 16)

        @block.sync
        def _(sp: bass.BassEngine):
            if in_dma == "sp":
                for rt in range(NRT):
                    sp.dma_start(
                        xt[rt][:], xm_ap[rt * P:(rt + 1) * P, :]
                    ).then_inc(in_sem[rt], 16)
            for t, rt, c_off, c_len in tiles:
                sp.wait_ge(act_sem, t + 1)
                sp.dma_start(
                    out_ap[rt * P:(rt + 1) * P,
                           c_off * K:(c_off + c_len) * K],
                    ot[t % nbuf_ot][:, 0:c_len * K],
                ).then_inc(od_sem[t % nbuf_ot], 16)

    return nc


_NC_CACHE = None


def _get_nc():
    global _NC_CACHE
    if _NC_CACHE is None:
        _NC_CACHE = build_nc(in_dma="sp")
    return _NC_CACHE


def pack_shard(x_shard: np.ndarray) -> np.ndarray:
    """[ROWS, F] f32 -> [ROWS, F+K] int32: sign-normalized bitcast columns
    plus the 32 mask columns."""
    x_shard = np.ascontiguousarray(x_shard)
    xi = x_shard.view(np.uint32)
    xi = (xi & np.uint32(0x7FFFFFFF)) | \
        ((x_shard < 0).astype(np.uint32) << np.uint32(31))
    m = np.broadcast_to(_masks_np(), (x_shard.shape[0], K))
    return np.ascontiguousarray(
        np.concatenate([xi.view(np.int32), m], axis=1))


def kernel(x: np.ndarray) -> np.ndarray:
    from concourse.bass_utils import run_bass_kernel_spmd

    x = np.asarray(x, dtype=np.float32)
    assert x.shape == (ROWS_TOTAL, F), x.shape
    nc = _get_nc()
    in_maps = [
        {"xm": pack_shard(x[i * ROWS:(i + 1) * ROWS])} for i in range(N_CORES)
    ]
    res = run_bass_kernel_spmd(nc, in_maps, list(range(N_CORES)))
    parts = [res.results[i]["out"].reshape(ROWS, F, K) for i in range(N_CORES)]
    return np.concatenate(parts, axis=0)
